# revision 1
# baseline (speedup 1.0000x reference)
"""Pendulum2 DAE kernel for Trainium2 (Bass/Tile), data-parallel over 8 cores.

Closed form per sample (coords = [x0 x1 x2 x3 v0 v1 v2 v3], M0=M1=G=10):
  d0 = x0-x2, d1 = x1-x3, w0 = v0-v2, w1 = v1-v3
  s1 = x0^2+x1^2, q = x0*d0+x1*d1, r = d0^2+d1^2
  h  = v0^2+v1^2 - 10*x1, k = w0^2+w1^2
  D  = 2*s1*r - q^2
  mu1 = (2*r*h - q*k)/D, mu2 = (s1*k - q*h)/D
  out = [v0 v1 v2 v3,
         -(x0*mu1+d0*mu2), -10-(x1*mu1+d1*mu2), d0*mu2, -10+d1*mu2]

v3 layout: all per-coordinate intermediates live as PACKED pairs
[(a[t],b[t]) interleaved, contiguous] so every op touching the
interleaved (t,e) input view reads 2-elem runs (half rate) instead of
1-elem columns (quarter rate), and everything else is full rate.
Pair sums are DVE tensor_reduce(axis=X) over contiguous packed pairs.
sqD uses scale=sqrt(2) so its pair-sum is r2=2r (kills both "2*"
scalings).  num1n = qk-2rh = -num1 makes mu1n = -mu1 so outputs are
plain subtracts.  1/D = exp(-ln(D)) on ACT (D >= s1*r > 0).  The
[0,10]/[0,-10] biases ride in via a tiny constant input broadcast
along t.
"""

import json

import numpy as np

from concourse import bass, bass_utils, mybir
from concourse.tile import TileContext


def _split_multi_waits(mod):
    # The walrus build here encodes at most one sync wait per instruction;
    # hoist extra waits onto wait-only EventSemaphore nops on the same engine
    # (in-order issue preserves semantics).
    ctr = 0
    for fn in mod.get("functions", []):
        for blk in fn.get("blocks", []):
            new = []
            for inst in blk.get("instructions", []):
                si = inst.get("sync_info") or {}
                ow = si.get("on_wait") or []
                if len(ow) > 1:
                    for w in ow[:-1]:
                        ctr += 1
                        new.append(
                            {
                                "debug": inst.get("debug", 0),
                                "engine": inst["engine"],
                                "ins": [],
                                "name": f"syncsplit-{ctr}-{inst['name']}",
                                "opcode": "EventSemaphore",
                                "outs": [],
                                "sync_info": {"on_wait": [w]},
                            }
                        )
                    si = dict(si)
                    si["on_wait"] = [ow[-1]]
                    inst = dict(inst)
                    inst["sync_info"] = si
                new.append(inst)
            blk["instructions"] = new
    return mod


_ORIG_TO_JSON_BYTES = bass.Bass.to_json_bytes


def _patched_to_json_bytes(self):
    return json.dumps(_split_multi_waits(json.loads(_ORIG_TO_JSON_BYTES(self)))).encode()


bass.Bass.to_json_bytes = _patched_to_json_bytes

BS = 2_097_152
NCORES = 8
PER = BS // NCORES          # samples per core
P = 128                     # SBUF partitions
T = 512                     # samples per partition-row per tile
NTILES = PER // (P * T)

f32 = mybir.dt.float32
ALU = mybir.AluOpType
ACTF = mybir.ActivationFunctionType
AXL = mybir.AxisListType
SQRT2 = float(np.sqrt(2.0))

# scratch plane map (26 planes, reused):
#  0-1  X01 packed          2-3  d01 packed
#  4-5  w01 packed -> U01 packed [x01*mu1n]
#  6-7  sqX packed -> an packed [d01*mu2]
#  8-9  sqV packed -> [mu1n, mu2]
# 10-11 sqD packed -> J [r2h, qk] -> anb packed
# 12-13 sqW packed -> K [qh, s1k]
# 14-15 m01 packed -> num1n, num2
# 16 r2  17 q  18 s1  19 h  20 k  21 t10  22 tq/lnD  23 s1r2/D
# 24 id  25 h1
NPLANES = 26


def _build():
    nc = bass.Bass()
    coords = nc.dram_tensor("coords", [PER, 8], f32, kind="ExternalInput")
    cbd = nc.dram_tensor("cb", [P, 4], f32, kind="ExternalInput")
    out = nc.dram_tensor("out", [PER, 8], f32, kind="ExternalOutput")

    cv = coords.rearrange("(n p t) e -> n p (t e)", n=NTILES, p=P, t=T)
    ov = out.rearrange("(n p t) e -> n p (t e)", n=NTILES, p=P, t=T)

    with TileContext(nc) as tc:
        with tc.tile_pool(name="cbp", bufs=1) as cbp, tc.tile_pool(
            name="io", bufs=2
        ) as iop, tc.tile_pool(name="sc", bufs=2) as scp:
            cbt = cbp.tile([P, 4], f32)
            nc.sync.dma_start(out=cbt, in_=cbd[:])
            cb01 = cbt[:, 0:2].rearrange("p (o e) -> p o e", o=1).broadcast_to((P, T, 2))
            cb23 = cbt[:, 2:4].rearrange("p (o e) -> p o e", o=1).broadcast_to((P, T, 2))

            for i in range(NTILES):
                in_t = iop.tile([P, T * 8], f32)
                out_t = iop.tile([P, T * 8], f32)
                sc = scp.tile([P, NPLANES * T], f32)

                nc.sync.dma_start(out=in_t, in_=cv[i])

                iv = in_t.rearrange("p (t e) -> p t e", e=8)
                ovt = out_t.rearrange("p (t e) -> p t e", e=8)
                s3 = sc.rearrange("p (pl t) -> p pl t", t=T)

                def fl(a, b):
                    return sc[:, a * T : b * T]

                def pk(a):
                    return fl(a, a + 2).rearrange("p (t e) -> p t e", e=2)

                def pl(j):
                    return sc[:, j * T : (j + 1) * T]

                def pls(a, b):
                    return s3[:, a:b]

                def bc(j):
                    return s3[:, j : j + 1].broadcast_to((P, 2, T))

                def bc2(j):
                    return pl(j).rearrange("p (t o) -> p t o", o=1).broadcast_to((P, T, 2))

                V, S, PO = nc.vector, nc.scalar, nc.gpsimd

                # stage x01/x23 packed; packed diffs/products off interleaved input
                S.copy(pk(0), iv[:, :, 0:2])                                   # X01
                S.copy(pk(10), iv[:, :, 2:4])                                  # X23
                V.tensor_sub(out=fl(2, 4), in0=fl(0, 2), in1=fl(10, 12))       # d01
                V.tensor_sub(out=pk(4), in0=iv[:, :, 4:6], in1=iv[:, :, 6:8])  # w01
                PO.tensor_tensor(out=fl(14, 16), in0=fl(0, 2), in1=fl(2, 4), op=ALU.mult)  # m01

                # squares (ACT, contiguous except sqV) + t10
                S.activation(fl(6, 8), fl(0, 2), ACTF.Square)                 # sqX
                S.activation(pk(8), iv[:, :, 4:6], ACTF.Square)               # sqV
                S.activation(fl(10, 12), fl(2, 4), ACTF.Square, scale=SQRT2)  # 2d^2
                S.activation(fl(12, 14), fl(4, 6), ACTF.Square)               # w^2
                S.activation(pl(21), pk(0)[:, :, 1], ACTF.Copy, scale=-10.0)  # t10

                # v passthrough (4-elem runs both sides, full rate)
                S.copy(ovt[:, :, 0:4], iv[:, :, 4:8])

                # pair sums
                PO.tensor_add(out=pl(18), in0=pk(6)[:, :, 0], in1=pk(6)[:, :, 1])    # s1
                PO.tensor_add(out=pl(20), in0=pk(12)[:, :, 0], in1=pk(12)[:, :, 1])  # k
                V.tensor_reduce(out=pl(16), in_=pk(10), axis=AXL.X, op=ALU.add)      # r2
                V.tensor_reduce(out=pl(25), in_=pk(8), axis=AXL.X, op=ALU.add)       # h1
                V.tensor_reduce(out=pl(17), in_=pk(14), axis=AXL.X, op=ALU.add)      # q
                V.tensor_add(out=pl(19), in0=pl(25), in1=pl(21))                     # h

                # products: tq=q^2, J=[r2h,qk], K=[qh,s1k], s1r2
                S.activation(pl(22), pl(17), ACTF.Square)
                V.tensor_tensor(out=pls(10, 12), in0=pls(16, 18), in1=pls(19, 21), op=ALU.mult)
                V.tensor_tensor(out=pls(12, 14), in0=pls(17, 19), in1=pls(19, 21), op=ALU.mult)
                V.tensor_tensor(out=pl(23), in0=pl(18), in1=pl(16), op=ALU.mult)

                # numerators / determinant / 1/D = exp(-ln D)
                PO.tensor_sub(out=pl(14), in0=pl(11), in1=pl(10))   # num1n = qk - 2rh
                PO.tensor_sub(out=pl(15), in0=pl(13), in1=pl(12))   # num2  = s1k - qh
                V.tensor_sub(out=pl(23), in0=pl(23), in1=pl(22))    # D
                S.activation(pl(22), pl(23), ACTF.Ln)
                S.activation(pl(24), pl(22), ACTF.Exp, scale=-1.0)  # id

                # mus and packed outputs
                V.tensor_tensor(out=pls(8, 10), in0=pls(14, 16), in1=bc(24), op=ALU.mult)  # [mu1n, mu2]
                V.tensor_tensor(out=pk(4), in0=pk(0), in1=bc2(8), op=ALU.mult)             # U01
                V.tensor_tensor(out=pk(6), in0=pk(2), in1=bc2(9), op=ALU.mult)             # an = [a2, n2]
                PO.tensor_add(out=fl(10, 12), in0=pk(6), in1=cb01)                         # anb = [a2, n2+10]
                PO.tensor_sub(out=ovt[:, :, 4:6], in0=pk(4), in1=pk(10))                   # [a0, a1]
                PO.tensor_tensor(out=ovt[:, :, 6:8], in0=pk(6), in1=cb23, op=ALU.add)      # [a2, a3]

                nc.sync.dma_start(out=ov[i], in_=out_t)
    return nc


_NC = None
_CB = None


def _run(coords, trace=False, **kw):
    global _NC, _CB
    if _NC is None:
        _NC = _build()
        _CB = np.tile(np.array([0.0, 10.0, 0.0, -10.0], dtype=np.float32), (P, 1))
    coords = np.ascontiguousarray(coords, dtype=np.float32)
    in_maps = [
        {"coords": coords[c * PER : (c + 1) * PER], "cb": _CB} for c in range(NCORES)
    ]
    res = bass_utils.run_bass_kernel_spmd(
        _NC, in_maps, core_ids=list(range(NCORES)), trace=trace, **kw
    )
    out = np.concatenate([res.results[c]["out"] for c in range(NCORES)], axis=0)
    return out, res


def kernel(t, coords):
    return _run(coords)[0]



# revision 14
# speedup vs baseline: 1.2697x; 1.2697x over previous
"""Pendulum2 DAE kernel for Trainium2 (Bass/Tile), data-parallel over 8 cores.

Closed form per sample (coords = [x0 x1 x2 x3 v0 v1 v2 v3], M0=M1=G=10):
  d0 = x0-x2, d1 = x1-x3, w0 = v0-v2, w1 = v1-v3
  s1 = x0^2+x1^2, q = x0*d0+x1*d1, r = d0^2+d1^2
  h  = v0^2+v1^2 - 10*x1, k = w0^2+w1^2
  D  = 2*s1*r - q^2
  mu1 = (2*r*h - q*k)/D, mu2 = (s1*k - q*h)/D
  out = [v0 v1 v2 v3,
         -(x0*mu1+d0*mu2), -10-(x1*mu1+d1*mu2), d0*mu2, -10+d1*mu2]

v6 design notes (from microbenchmark calibration + trace analysis):
 - Creation order IS the dependency order Tile tracks: every instruction is
   created after its producers.
 - Strided / broadcast APs are free on DVE and ACT; ops read the interleaved
   (t e) input view directly (no packing copies).
 - GpSimd shares the DVE SBUF port and mislowers 3-level strided views, so it
   only gets the two cb-broadcast bias adds (baseline-proven patterns).
 - All five pair-sums run as ONE fused DVE TT over planes 4-13 (even vs odd
   elements), landing [r2, q, s1, h1, k] in planes 14-18.
 - sqD uses scale=sqrt(2) so its pair-sum is r2=2r; num1n = qk-2rh = -mu1*D
   makes the output stage plain subtracts; 1/D = exp(-ln(D)) (D >= s1*r > 0).
 - Asymmetric tiles [256, 512, 512, 512, 256]: small first tile cuts the DMA
   fill latency, small last tile cuts the drain.
 - prods/nums are created before the tq/D/ln/exp chain so they overlap it.
"""

import json

import numpy as np

from concourse import bass, bass_utils, mybir
from concourse.tile import TileContext


def _split_multi_waits(mod):
    # walrus encodes at most one sync wait per instruction; hoist extra waits
    # onto wait-only EventSemaphore nops on the same engine (in-order issue
    # preserves semantics).
    ctr = 0
    for fn in mod.get("functions", []):
        for blk in fn.get("blocks", []):
            new = []
            for inst in blk.get("instructions", []):
                si = inst.get("sync_info") or {}
                ow = si.get("on_wait") or []
                if len(ow) > 1:
                    for w in ow[:-1]:
                        ctr += 1
                        new.append(
                            {
                                "debug": inst.get("debug", 0),
                                "engine": inst["engine"],
                                "ins": [],
                                "name": f"syncsplit-{ctr}-{inst['name']}",
                                "opcode": "EventSemaphore",
                                "outs": [],
                                "sync_info": {"on_wait": [w]},
                            }
                        )
                    si = dict(si)
                    si["on_wait"] = [ow[-1]]
                    inst = dict(inst)
                    inst["sync_info"] = si
                new.append(inst)
            blk["instructions"] = new
    return mod


_ORIG_TO_JSON_BYTES = bass.Bass.to_json_bytes


def _patched_to_json_bytes(self):
    return json.dumps(_split_multi_waits(json.loads(_ORIG_TO_JSON_BYTES(self)))).encode()


bass.Bass.to_json_bytes = _patched_to_json_bytes

BS = 2_097_152
NCORES = 8
PER = BS // NCORES          # samples per core
P = 128                     # SBUF partitions
TMAX = 512
TILES = [256, 512, 512, 512, 256]   # samples per partition-row per tile
assert sum(TILES) * P == PER

f32 = mybir.dt.float32
ALU = mybir.AluOpType
ACTF = mybir.ActivationFunctionType
SQRT2 = float(np.sqrt(2.0))

# scratch plane map (TMAX floats each; only the first tt of each used):
#  0-1  d01 pk      2-3  w01 pk -> [mu1n|mu2]
#  4-5  sqD pk -> an pk         6-7  m01 pk -> anb pk
#  8-9  sqX pk -> U01 pk       10-11 sqV pk   12-13 sqW pk
# fused pair-sum TT over planes 4-13 -> planes 14-18 = [r2, q, s1, h1, k]
# 19 tq  20 s1r2/D  21 lnD  22 invD
# 23-24 [r2h|qh]   25-26 [qk|s1k]   27-28 [num1n|num2]
NPLANES = 29


def _build():
    nc = bass.Bass()
    coords = nc.dram_tensor("coords", [PER, 8], f32, kind="ExternalInput")
    cbd = nc.dram_tensor("cb", [P, 4], f32, kind="ExternalInput")
    out = nc.dram_tensor("out", [PER, 8], f32, kind="ExternalOutput")

    with TileContext(nc) as tc:
        with tc.tile_pool(name="cbp", bufs=1) as cbp, tc.tile_pool(
            name="io", bufs=2
        ) as iop, tc.tile_pool(name="sc", bufs=2) as scp:
            cbt = cbp.tile([P, 4], f32)
            nc.sync.dma_start(out=cbt, in_=cbd[:])

            off = 0
            for tt in TILES:
                dram_in = coords[off : off + P * tt].rearrange("(p t) e -> p (t e)", p=P)
                dram_out = out[off : off + P * tt].rearrange("(p t) e -> p (t e)", p=P)
                off += P * tt

                in_full = iop.tile([P, TMAX * 8], f32)
                out_full = iop.tile([P, TMAX * 8], f32)
                sc = scp.tile([P, NPLANES * TMAX], f32)
                in_t = in_full[:, : tt * 8]
                out_t = out_full[:, : tt * 8]

                nc.sync.dma_start(out=in_t, in_=dram_in)

                iv = in_t.rearrange("p (t e) -> p t e", e=8)
                ovt = out_t.rearrange("p (t e) -> p t e", e=8)

                def pk(a):
                    return sc[:, a * TMAX : a * TMAX + 2 * tt].rearrange(
                        "p (t e) -> p t e", e=2
                    )

                def pl(j):
                    return sc[:, j * TMAX : j * TMAX + tt]

                def pls(a, b):
                    return sc[:, a * TMAX : b * TMAX].rearrange(
                        "p (c t) -> p c t", t=TMAX
                    )[:, :, :tt]

                def bco(j):
                    return pl(j).rearrange("p (o t) -> p o t", o=1).broadcast_to((P, 2, tt))

                def bc2(j):
                    return pl(j).rearrange("p (t o) -> p t o", o=1).broadcast_to((P, tt, 2))

                cb01 = cbt[:, 0:2].rearrange("p (o e) -> p o e", o=1).broadcast_to((P, tt, 2))
                cb23 = cbt[:, 2:4].rearrange("p (o e) -> p o e", o=1).broadcast_to((P, tt, 2))

                V, S, G = nc.vector, nc.scalar, nc.gpsimd

                # diffs + squares
                V.tensor_sub(out=pk(0), in0=iv[:, :, 0:2], in1=iv[:, :, 2:4])   # d01
                V.tensor_sub(out=pk(2), in0=iv[:, :, 4:6], in1=iv[:, :, 6:8])   # w01
                S.activation(pk(4), pk(0), ACTF.Square, scale=SQRT2)            # sqD = 2d^2
                S.activation(pk(8), iv[:, :, 0:2], ACTF.Square)                 # sqX
                S.activation(pk(10), iv[:, :, 4:6], ACTF.Square)                # sqV
                S.activation(pk(12), pk(2), ACTF.Square)                        # sqW
                S.copy(ovt[:, :, 0:4], iv[:, :, 4:8])                           # v passthrough
                V.tensor_tensor(out=pk(6), in0=iv[:, :, 0:2], in1=pk(0), op=ALU.mult)  # m01

                # all five pair-sums in one TT: planes 4-13 even vs odd elems
                sq5 = (
                    sc[:, 4 * TMAX : 14 * TMAX]
                    .rearrange("p (c r) -> p c r", c=5)[:, :, : 2 * tt]
                    .rearrange("p c (t e) -> p c t e", e=2)
                )
                fl_ps = sc[:, 14 * TMAX : 19 * TMAX].rearrange("p (c t) -> p c t", c=5)[
                    :, :, :tt
                ]
                V.tensor_add(out=fl_ps, in0=sq5[:, :, :, 0], in1=sq5[:, :, :, 1])  # [r2,q,s1,h1,k]

                # h = h1 - 10*x1, then numerator products (independent of D-chain)
                V.scalar_tensor_tensor(
                    out=pl(17), in0=iv[:, :, 1], scalar=-10.0, in1=pl(17),
                    op0=ALU.mult, op1=ALU.add,
                )
                V.tensor_tensor(out=pls(23, 25), in0=pls(14, 16), in1=bco(17), op=ALU.mult)  # [r2h|qh]
                V.tensor_tensor(out=pls(25, 27), in0=pls(15, 17), in1=bco(18), op=ALU.mult)  # [qk|s1k]
                V.tensor_sub(out=pls(27, 29), in0=pls(25, 27), in1=pls(23, 25))   # [num1n|num2]

                # D chain
                V.tensor_tensor(out=pl(19), in0=pl(15), in1=pl(15), op=ALU.mult)  # tq = q^2
                V.tensor_tensor(out=pl(20), in0=pl(16), in1=pl(14), op=ALU.mult)  # s1*r2
                V.tensor_sub(out=pl(20), in0=pl(20), in1=pl(19))                  # D
                S.activation(pl(21), pl(20), ACTF.Ln)                           # ln D
                S.activation(pl(22), pl(21), ACTF.Exp, scale=-1.0)              # invD
                V.tensor_tensor(out=pls(2, 4), in0=pls(27, 29), in1=bco(22), op=ALU.mult)  # [mu1n|mu2]

                # combine
                V.tensor_tensor(out=pk(4), in0=pk(0), in1=bc2(3), op=ALU.mult)    # an = d01*mu2
                G.tensor_tensor(out=ovt[:, :, 6:8], in0=pk(4), in1=cb23, op=ALU.add)  # [a2, a3]
                G.tensor_tensor(out=pk(6), in0=pk(4), in1=cb01, op=ALU.add)       # anb
                V.tensor_tensor(out=pk(8), in0=iv[:, :, 0:2], in1=bc2(2), op=ALU.mult)  # U01
                V.tensor_sub(out=ovt[:, :, 4:6], in0=pk(8), in1=pk(6))            # [a0, a1]

                nc.sync.dma_start(out=dram_out, in_=out_t)
    return nc


_NC = None
_CB = None


def _run(coords, trace=False, **kw):
    global _NC, _CB
    if _NC is None:
        _NC = _build()
        _CB = np.tile(np.array([0.0, 10.0, 0.0, -10.0], dtype=np.float32), (P, 1))
    coords = np.ascontiguousarray(coords, dtype=np.float32)
    in_maps = [
        {"coords": coords[c * PER : (c + 1) * PER], "cb": _CB} for c in range(NCORES)
    ]
    res = bass_utils.run_bass_kernel_spmd(
        _NC, in_maps, core_ids=list(range(NCORES)), trace=trace, **kw
    )
    out = np.concatenate([res.results[c]["out"] for c in range(NCORES)], axis=0)
    return out, res


def kernel(t, coords):
    return _run(coords)[0]
